# revision 3
# baseline (speedup 1.0000x reference)
"""Trainium2 Bass kernel for nn_KernelShiftedPrediction.

For each pixel, over 9 shifts (x,y) in {-1,0,1}^2 (priority order:
(0,0) first, then row-major), pick the shifted `predicted` value
minimizing |target - candidate| with strict first-occurrence
tie-breaking; out-of-bounds shifts never win.

Sharding: batch dim B=8 -> 8 NeuronCores (embarrassingly parallel).
Per core: 10 images of [512,512], processed in pairs of images x 4
row-chunks of 128 partitions. Vertical shifts come from three
row-shifted DMA views (up/center/down); horizontal shifts are free-dim
AP offsets into column-padded (1e30) view tiles.

Engine split per shift: GPSIMD tensor_tensor subtract, ACT activation
Abs, DVE is_lt + min + copy_predicated (exact running argmin).
"""
import sys

sys.path.insert(0, "/opt/trn_rl_repo")

import numpy as np

S, B, H, W = 10, 8, 512, 512
PAIR = 2          # images per tile group
NPAIR = S // PAIR
CH = 128          # chunk rows (partitions)
NCH = H // CH
SEG = W + 2       # per-image segment width in padded view tiles
FREE_T = PAIR * W
FREE_P = PAIR * SEG
PADVAL = 1.0e30

# priority order after the (0,0) seed; duplicate (0,0) skipped (strict <)
SHIFTS = [(-1, -1), (-1, 0), (-1, 1), (0, -1), (0, 1), (1, -1), (1, 0), (1, 1)]

_CACHE = {}


def _build_nc():
    import concourse.bacc as bacc
    import concourse.mybir as mybir
    from concourse.tile import TileContext

    F32 = mybir.dt.float32
    U8 = mybir.dt.uint8
    OP = mybir.AluOpType
    ABS = mybir.ActivationFunctionType.Abs

    nc = bacc.Bacc("TRN2", target_bir_lowering=False, debug=False, num_devices=B)
    pred = nc.declare_dram_parameter("pred", [S, H, W], F32, isOutput=False)
    targ = nc.declare_dram_parameter("targ", [S, H, W], F32, isOutput=False)
    out = nc.declare_dram_parameter("out", [S, H, W], F32, isOutput=True)

    with TileContext(nc) as tc:
        with (
            tc.tile_pool(name="io", bufs=3) as io,
            tc.tile_pool(name="wk", bufs=2) as wk,
            tc.tile_pool(name="mk", bufs=2) as mk,
        ):
            for p in range(NPAIR):
                s0 = p * PAIR
                for c in range(NCH):
                    r0 = c * CH
                    T = io.tile([CH, FREE_T], F32, tag="T")
                    PU = io.tile([CH, FREE_P], F32, tag="PU")
                    PC = io.tile([CH, FREE_P], F32, tag="PC")
                    PD = io.tile([CH, FREE_P], F32, tag="PD")

                    # column pads (both edges of each image segment)
                    for V in (PU, PC, PD):
                        for k in range(PAIR):
                            nc.gpsimd.memset(
                                V[:, k * SEG : k * SEG + SEG : SEG - 1], PADVAL
                            )

                    for k in range(PAIR):
                        s = s0 + k
                        cs = k * SEG + 1  # data col start in view tiles
                        nc.sync.dma_start(
                            out=T[:, k * W : (k + 1) * W],
                            in_=targ[s, r0 : r0 + CH, :],
                        )
                        nc.sync.dma_start(
                            out=PC[:, cs : cs + W], in_=pred[s, r0 : r0 + CH, :]
                        )
                        if c == 0:
                            nc.vector.memset(PU[0:32, k * SEG : (k + 1) * SEG], PADVAL)
                            nc.sync.dma_start(
                                out=PU[1:CH, cs : cs + W],
                                in_=pred[s, 0 : CH - 1, :],
                            )
                        else:
                            nc.sync.dma_start(
                                out=PU[:, cs : cs + W],
                                in_=pred[s, r0 - 1 : r0 + CH - 1, :],
                            )
                        if c == NCH - 1:
                            nc.vector.memset(
                                PD[96:CH, k * SEG : (k + 1) * SEG], PADVAL
                            )
                            nc.sync.dma_start(
                                out=PD[0 : CH - 1, cs : cs + W],
                                in_=pred[s, r0 + 1 : H, :],
                            )
                        else:
                            nc.sync.dma_start(
                                out=PD[:, cs : cs + W],
                                in_=pred[s, r0 + 1 : r0 + CH + 1, :],
                            )

                    VX = {-1: PU, 0: PC, 1: PD}

                    def cand(x, y):
                        v = VX[x][:, :].rearrange("p (s w) -> p s w", s=PAIR)
                        return v[:, :, 1 + y : 1 + y + W]

                    T3 = T[:, :].rearrange("p (s w) -> p s w", s=PAIR)

                    d = wk.tile([CH, FREE_T], F32, tag="d")
                    l = wk.tile([CH, FREE_T], F32, tag="l")
                    bl = wk.tile([CH, FREE_T], F32, tag="bl")
                    bv = wk.tile([CH, FREE_T], F32, tag="bv")
                    m = mk.tile([CH, FREE_T], U8, tag="m")
                    d3 = d[:, :].rearrange("p (s w) -> p s w", s=PAIR)
                    l3 = l[:, :].rearrange("p (s w) -> p s w", s=PAIR)
                    bl3 = bl[:, :].rearrange("p (s w) -> p s w", s=PAIR)
                    bv3 = bv[:, :].rearrange("p (s w) -> p s w", s=PAIR)
                    m3 = m[:, :].rearrange("p (s w) -> p s w", s=PAIR)

                    # seed with (0,0)
                    c00 = cand(0, 0)
                    nc.gpsimd.tensor_tensor(d3, T3, c00, OP.subtract)
                    nc.scalar.activation(bl3, d3, ABS)
                    nc.scalar.copy(bv3, c00)

                    for (x, y) in SHIFTS:
                        cxy = cand(x, y)
                        nc.gpsimd.tensor_tensor(d3, T3, cxy, OP.subtract)
                        nc.scalar.activation(l3, d3, ABS)
                        nc.vector.tensor_tensor(m3, l3, bl3, OP.is_lt)
                        nc.vector.tensor_tensor(bl3, l3, bl3, OP.min)
                        nc.vector.copy_predicated(bv3, m3, cxy)

                    for k in range(PAIR):
                        nc.sync.dma_start(
                            out=out[s0 + k, r0 : r0 + CH, :],
                            in_=bv[:, k * W : (k + 1) * W],
                        )
    nc.finalize()
    return nc


def _get_nc():
    if "nc" not in _CACHE:
        _CACHE["nc"] = _build_nc()
    return _CACHE["nc"]


def kernel(predicted, target, mask=None, _want_results_obj=False, _trace=False):
    """predicted [S,B,H,W], target [B,S,H,W] -> [S,B,H,W] (mask unused)."""
    from concourse.bass_utils import run_bass_kernel_spmd

    nc = _get_nc()
    in_maps = []
    for b in range(B):
        in_maps.append(
            {
                "pred": np.ascontiguousarray(predicted[:, b]),
                "targ": np.ascontiguousarray(target[b]),
            }
        )
    res = run_bass_kernel_spmd(nc, in_maps, list(range(B)), trace=_trace)
    outp = np.stack([res.results[b]["out"] for b in range(B)], axis=1)
    if _want_results_obj:
        return outp, res
    return outp


# revision 4
# speedup vs baseline: 1.5047x; 1.5047x over previous
"""Trainium2 Bass kernel for nn_KernelShiftedPrediction (v3).

For each pixel, over 9 shifts (x,y) in {-1,0,1}^2 (priority order:
(0,0) first, then row-major), pick the shifted `predicted` value
minimizing |target - candidate| with strict first-occurrence
tie-breaking; out-of-bounds shifts never win (1e30 padding).

Sharding: batch dim B=8 -> 8 NeuronCores. Per core: 10 images of
[512,512]; one image per iteration, its 4 row-chunks of 128 rows laid
side-by-side in the free dim (N=2048). Vertical shifts come from three
row-shifted DMA views (up/center/down); horizontal shifts are free-dim
AP offsets into column-padded view tiles.

Engine split (avoids the GPSIMD<->DVE shared-SBUF-port contention):
 - PE: d_s = I@T + (-I)@C_s accumulated in PSUM (bit-exact fp32)
 - ACT: l_s = Abs(PSUM d_s) -> SBUF; also seeds bl/bv
 - DVE: is_lt + min + copy_predicated (exact running argmin), 24 ops/img
 - GPSIMD: only tiny pad memsets
"""
import sys

sys.path.insert(0, "/opt/trn_rl_repo")

import numpy as np

S, B, H, W = 10, 8, 512, 512
CH = 128          # chunk rows (partitions)
NCH = H // CH     # 4 segments (row-chunks) per image, side by side
SEG = W + 2       # per-segment width in padded view tiles
FREE_T = NCH * W      # 2048
FREE_P = NCH * SEG    # 2056
PADVAL = 1.0e30
MMW = 512         # matmul free width (one PSUM bank)

# priority order after the (0,0) seed; duplicate (0,0) skipped (strict <)
SHIFTS = [(-1, -1), (-1, 0), (-1, 1), (0, -1), (0, 1), (1, -1), (1, 0), (1, 1)]

_CACHE = {}


def _build_nc():
    import concourse.bacc as bacc
    import concourse.mybir as mybir
    from concourse.tile import TileContext

    F32 = mybir.dt.float32
    U8 = mybir.dt.uint8
    OP = mybir.AluOpType
    ABS = mybir.ActivationFunctionType.Abs

    nc = bacc.Bacc("TRN2", target_bir_lowering=False, debug=False, num_devices=B)
    pred = nc.declare_dram_parameter("pred", [S, H, W], F32, isOutput=False)
    targ = nc.declare_dram_parameter("targ", [S, H, W], F32, isOutput=False)
    eye2 = nc.declare_dram_parameter("eye2", [128, 256], F32, isOutput=False)
    out = nc.declare_dram_parameter("out", [S, H, W], F32, isOutput=True)

    with TileContext(nc) as tc:
        with (
            tc.tile_pool(name="cst", bufs=1) as cst,
            tc.tile_pool(name="io", bufs=2) as io,
            tc.tile_pool(name="wk", bufs=2) as wk,
            tc.tile_pool(name="mk", bufs=2) as mk,
            tc.tile_pool(name="ps", bufs=8, space="PSUM") as psp,
        ):
            eye = cst.tile([128, 256], F32)
            nc.sync.dma_start(out=eye[:, :], in_=eye2[:, :])
            W_I = eye[:, 0:128]     # identity
            W_N = eye[:, 128:256]   # -identity

            for s in range(S):
                T = io.tile([CH, FREE_T], F32, tag="T")
                PU = io.tile([CH, FREE_P], F32, tag="PU")
                PC = io.tile([CH, FREE_P], F32, tag="PC")
                PD = io.tile([CH, FREE_P], F32, tag="PD")

                # column pads: both edges of every segment, one memset per view
                for V in (PU, PC, PD):
                    ap = V[:, :].rearrange("p (g e) -> p g e", g=NCH)
                    nc.gpsimd.memset(ap[:, :, 0:SEG:SEG - 1], PADVAL)

                # row-edge pads (set before DMAs partially overwrite)
                nc.vector.memset(PU[0:32, 0:SEG], PADVAL)
                nc.vector.memset(PD[96:CH, (NCH - 1) * SEG : NCH * SEG], PADVAL)

                for g in range(NCH):
                    r0 = g * CH
                    cs = g * SEG + 1
                    nc.sync.dma_start(
                        out=T[:, g * W : (g + 1) * W], in_=targ[s, r0 : r0 + CH, :]
                    )
                    nc.sync.dma_start(
                        out=PC[:, cs : cs + W], in_=pred[s, r0 : r0 + CH, :]
                    )
                    if g == 0:
                        nc.sync.dma_start(
                            out=PU[1:CH, cs : cs + W], in_=pred[s, 0 : CH - 1, :]
                        )
                    else:
                        nc.sync.dma_start(
                            out=PU[:, cs : cs + W],
                            in_=pred[s, r0 - 1 : r0 + CH - 1, :],
                        )
                    if g == NCH - 1:
                        nc.sync.dma_start(
                            out=PD[0 : CH - 1, cs : cs + W], in_=pred[s, r0 + 1 : H, :]
                        )
                    else:
                        nc.sync.dma_start(
                            out=PD[:, cs : cs + W],
                            in_=pred[s, r0 + 1 : r0 + CH + 1, :],
                        )

                VX = {-1: PU, 0: PC, 1: PD}

                def cand(x, y):
                    v = VX[x][:, :].rearrange("p (g w) -> p g w", g=NCH)
                    return v[:, :, 1 + y : 1 + y + W]

                def cand_seg(x, y, g):
                    return VX[x][:, g * SEG + 1 + y : g * SEG + 1 + y + W]

                l = wk.tile([CH, FREE_T], F32, tag="l")
                bl = wk.tile([CH, FREE_T], F32, tag="bl")
                bv = wk.tile([CH, FREE_T], F32, tag="bv")
                m = mk.tile([CH, FREE_T], U8, tag="m")

                def g3(t):
                    return t[:, :].rearrange("p (g w) -> p g w", g=NCH)

                # seed with (0,0): bl = |T - PC|, bv = PC  (PE + ACT)
                for g in range(NCH):
                    ps = psp.tile([CH, MMW], F32, tag="ps")
                    nc.tensor.matmul(
                        ps[:, :], W_I, T[:, g * W : (g + 1) * W],
                        start=True, stop=False,
                    )
                    nc.tensor.matmul(
                        ps[:, :], W_N, cand_seg(0, 0, g), start=False, stop=True
                    )
                    nc.scalar.activation(bl[:, g * W : (g + 1) * W], ps[:, :], ABS)
                nc.scalar.copy(g3(bv), cand(0, 0))

                for (x, y) in SHIFTS:
                    for g in range(NCH):
                        ps = psp.tile([CH, MMW], F32, tag="ps")
                        nc.tensor.matmul(
                            ps[:, :], W_I, T[:, g * W : (g + 1) * W],
                            start=True, stop=False,
                        )
                        nc.tensor.matmul(
                            ps[:, :], W_N, cand_seg(x, y, g), start=False, stop=True
                        )
                        nc.scalar.activation(l[:, g * W : (g + 1) * W], ps[:, :], ABS)
                    nc.vector.tensor_tensor(m[:, :], l[:, :], bl[:, :], OP.is_lt)
                    nc.vector.tensor_tensor(bl[:, :], l[:, :], bl[:, :], OP.min)
                    nc.vector.copy_predicated(g3(bv), g3(m), cand(x, y))

                for g in range(NCH):
                    nc.sync.dma_start(
                        out=out[s, g * CH : (g + 1) * CH, :],
                        in_=bv[:, g * W : (g + 1) * W],
                    )
    nc.finalize()
    return nc


def _get_nc():
    if "nc" not in _CACHE:
        _CACHE["nc"] = _build_nc()
    return _CACHE["nc"]


def kernel(predicted, target, mask=None, _want_results_obj=False, _trace=False):
    """predicted [S,B,H,W], target [B,S,H,W] -> [S,B,H,W] (mask unused)."""
    from concourse.bass_utils import run_bass_kernel_spmd

    nc = _get_nc()
    eye = np.eye(128, dtype=np.float32)
    eye2 = np.concatenate([eye, -eye], axis=1)
    in_maps = []
    for b in range(B):
        in_maps.append(
            {
                "pred": np.ascontiguousarray(predicted[:, b]),
                "targ": np.ascontiguousarray(target[b]),
                "eye2": eye2,
            }
        )
    res = run_bass_kernel_spmd(nc, in_maps, list(range(B)), trace=_trace)
    outp = np.stack([res.results[b]["out"] for b in range(B)], axis=1)
    if _want_results_obj:
        return outp, res
    return outp
